# revision 1
# baseline (speedup 1.0000x reference)
"""Trainium2 Bass kernel for nn_DrawImageLayer (draw Gaussian strokes, max over time).

Reference semantics:
  out[b,i,j,0] = min(1, max_t I[b,t] * exp(-g*(r_i - y[b,t])^2) * exp(-g*(r_j - x[b,t])^2))
  r_k = k/28 - 0.5, g = (28/2)^2 = 196, shapes B=1024, T=64, canvas 28x28.

Strategy: pure data parallel, 128 batch rows per NeuronCore (= SBUF
partitions) across 8 cores. Log domain so exp commutes with max:
  out = exp( max_t [ (lnI[t] - q_x[t,j]) - q_y[t,i] ] ),  q = g*(r-coord)^2
The min(.,1) clamp is dropped: I < 1 strictly => all log values < 0.

Cost model measured on this runtime (microbench*.py / bisect.py): every
dependent engine instruction costs a large fixed amount (~30-80us) nearly
independent of element count; blocking semaphore waits add ~25-40us; work on
different engines overlaps only when independent; DMA-in ~2.5us but
engine-issued DMAs and same-destination out-DMA bursts are expensive;
tensor_reduce is far cheaper with axis=XY and a small outer t-factor
(t=2x32: ~90us saved vs axis=X over t=64). Hence: fewest possible dependent
instructions, cheap prep on the Pool engine, one fused fp16 cube, one
XY-reduce, deep multi-buffering so every wait on the critical path is
pre-satisfied, and in-DMA prefetch ahead of the blocking out-DMA on the sync
queue.

Per rep (per core), instruction list:
  sync : in-dma xs[k%BUF] (prefetched BUF=5 ahead; WAR wait pre-satisfied)
  pool : d12 = r' - coord'   (fp16, 3584)   } independent of DVE, overlaps
         q   = d12 * d12     (in-place)     } the previous rep's DVE work
         ex  = lnI - q_x     (fp16, 1792)   }
  dve  : cube[i,j,t] = ex - q_y   one fp16 50176-elem TT (2x mode)
         img[k%4] = max_t cube    one XY-reduce over t = (2, 32)
  act  : imgo[k%4] = Exp(img)     (the only ACT op)
  sync : out-dma imgo -> out
"""

from contextlib import ExitStack

import numpy as np

import concourse.bass as bass
import concourse.mybir as mybir
from concourse.bass_utils import run_bass_kernel_spmd

SIZE = 28
T = 64
B = 1024
BC = 128  # batch rows per core
NCORES = 8
P2 = SIZE * SIZE
G = (SIZE / 2.0) ** 2
F32 = mybir.dt.float32
F16 = mybir.dt.float16
AO = mybir.AluOpType
AF = mybir.ActivationFunctionType

XCOLS = 3 * T + SIZE + 1  # y(64) | x(64) | lnI(64) | r(28) | e, t innermost
D12 = 2 * T * SIZE  # 3584, layout (c, k, t)
EXN = SIZE * T  # 1792, layout (j, t)
CUBE = P2 * T  # 50176, layout (i, j, t), t innermost
IH = SIZE // 2  # image rows per reduce (14 -> 392 output segments)
BUF = 4  # buffer depth; re-tuned after APMIN+EXMERGE shrank the cycle
PREC16 = True  # d12/ex in fp16: frees SBUF for deeper BUF + all-16bit cube
RED2 = False  # two 392-segment reduces instead of one 784-segment
RED_XY = True  # reduce axis XY over (t_hi, t_lo) instead of X over t
RED_SPLIT = (2, 32)  # (t_hi, t_lo); coupled to BUF: (2,32) wins at BUF=4, (8,8) at BUF=5
DUMMY_ACT = False  # pad ACT stream so Exp's vrd wait is pre-satisfied
CUBE_DT = F16  # cube dtype
IMG_DT = F32  # reduce output dtype
ALLDVE = False  # prep on DVE (no pool engine, no gex handoff)
APMIN = True  # merged-dim APs for cube in0 and reduce segments (~2x faster uop path)
EXMERGE = True  # merge ex's q_x input to one contiguous dim

_GRID = (np.arange(SIZE, dtype=np.float32) / SIZE - 0.5).astype(np.float32)


def _ap(t, offset, dims):
    """AP over an sbuf tensor: partition dim [row_pitch, 128] + free dims."""
    return bass.AP(t, offset, [[t.shape[1], BC]] + [list(d) for d in dims])


def build(rep: int = 1) -> bass.Bass:
    nc = bass.Bass()
    xin = nc.declare_dram_parameter("xin", [BC, XCOLS], F32, isOutput=False)
    out = nc.declare_dram_parameter("out", [BC, P2], F32, isOutput=True)

    with ExitStack() as ctx:
        # triple-buffered small tensors (index k%BUF) so the pool engine can
        # run 2 reps ahead of DVE and every DVE-path wait is pre-satisfied
        pdt = F16 if PREC16 else F32
        xs = ctx.enter_context(nc.sbuf_tensor([BC, BUF * XCOLS], F32))
        d12 = ctx.enter_context(nc.sbuf_tensor([BC, BUF * D12], pdt))
        ex = ctx.enter_context(nc.sbuf_tensor([BC, BUF * EXN], pdt))
        cube = ctx.enter_context(nc.sbuf_tensor([BC, CUBE], CUBE_DT))
        img = ctx.enter_context(nc.sbuf_tensor([BC, BUF * P2], IMG_DT))
        imgo = ctx.enter_context(nc.sbuf_tensor([BC, BUF * P2], F32))
        dsx = ctx.enter_context(nc.semaphore("dsx"))  # in-dma done
        gex = ctx.enter_context(nc.semaphore("gex"))  # pool ex done
        vrd = ctx.enter_context(nc.semaphore("vrd"))  # dve reduces done
        aex = ctx.enter_context(nc.semaphore("aex"))  # act exp done
        dso = ctx.enter_context(nc.semaphore("dso"))  # out-dma done
        block = ctx.enter_context(nc.Block())

        def in_dma(sync, k):
            di = sync.dma_start(
                out=_ap(xs, (k % BUF) * XCOLS, [[1, XCOLS]]), in_=xin[:, :]
            )
            if k > BUF - 1:
                # WAR with BUF-rep slack: Exp(k-BUF) done => red/cube/pool
                # (k-BUF) done => xs/d12/ex/img[k%BUF] all consumed
                di._wait_ge(aex, k - BUF + 1)
            di.then_inc(dsx, 16)

        @block.sync
        def _(sync):
            # issue in-dma(k+BUF) BEFORE out-dma(k): the out-dma's blocking
            # wait must not starve the pool engine of the next rep's input
            for k in range(min(rep, BUF)):
                in_dma(sync, k)
            for k in range(rep):
                if k + BUF < rep:
                    in_dma(sync, k + BUF)
                sync.dma_start(
                    out=out[:, :], in_=_ap(imgo, (k % BUF) * P2, [[1, P2]])
                )._wait_ge(aex, k + 1).then_inc(dso, 16)
            sync.wait_ge(dsx, rep * 16)
            sync.wait_ge(dso, rep * 16)

        def build_prep(eng, is_dve):
            def prep(k):
                o = (k % BUF) * XCOLS
                od = (k % BUF) * D12
                # d12[(c,kk,t)] = sqrt(g)*(r_kk - coord_c[t])  (host pre-scales)
                eng.tensor_tensor(
                    _ap(d12, od, [[1, D12]]),
                    _ap(xs, o + 3 * T, [[0, 2], [1, SIZE], [0, T]]),
                    _ap(xs, o, [[T, 2], [0, SIZE], [1, T]]),
                    AO.subtract,
                )._wait_ge(dsx, k * 16 + 16)
                # q = d12 * d12 = g*(r-coord)^2, in place
                if is_dve:
                    eng.scalar_tensor_tensor(
                        _ap(d12, od, [[1, D12]]),
                        _ap(d12, od, [[1, D12]]),
                        1.0,
                        _ap(d12, od, [[1, D12]]),
                        AO.mult,
                        AO.mult,
                    )
                else:
                    eng.tensor_tensor(
                        _ap(d12, od, [[1, D12]]),
                        _ap(d12, od, [[1, D12]]),
                        _ap(d12, od, [[1, D12]]),
                        AO.mult,
                    )
                # ex[(j,t)] = lnI[t] - q_x[(j,t)]; q_x (j,t) is contiguous
                e = eng.tensor_tensor(
                    _ap(ex, (k % BUF) * EXN, [[1, EXN]]),
                    _ap(xs, o + 2 * T, [[0, SIZE], [1, T]]),
                    _ap(d12, od + T * SIZE, [[1, EXN]] if EXMERGE else [[T, SIZE], [1, T]]),
                    AO.subtract,
                )
                if not is_dve:
                    e.then_inc(gex, 1)

            return prep

        if not ALLDVE:

            @block.gpsimd
            def _(gpsimd):
                prep = build_prep(nc.gpsimd, False)
                for k in range(rep):
                    prep(k)

        @block.vector
        def _(vector):
            dve_prep = build_prep(nc.vector, True) if ALLDVE else None
            for k in range(rep):
                od = (k % BUF) * D12
                if ALLDVE:
                    dve_prep(k)
                # cube[(i,j,t)] = ex[(j,t)] - q_y[(i,t)]
                ex_dims = (
                    [[0, SIZE], [1, EXN]] if APMIN
                    else [[0, SIZE], [T, SIZE], [1, T]]
                )
                cb = nc.vector.tensor_tensor(
                    _ap(cube, 0, [[1, CUBE]]),
                    _ap(ex, (k % BUF) * EXN, ex_dims),
                    _ap(d12, od, [[T, SIZE], [0, SIZE], [1, T]]),
                    AO.subtract,
                )
                if not ALLDVE:
                    cb._wait_ge(gex, k + 1)
                if RED2:
                    for h in range(2):
                        red = nc.vector.tensor_reduce(
                            _ap(img, (k % BUF) * P2 + h * IH * SIZE, [[1, IH * SIZE]]),
                            _ap(cube, h * IH * SIZE * T, [[SIZE * T, IH], [T, SIZE], [1, T]]),
                            mybir.AxisListType.X,
                            AO.max,
                        )
                elif RED_XY and APMIN:
                    th, tl = RED_SPLIT
                    red = nc.vector.tensor_reduce(
                        _ap(img, (k % BUF) * P2, [[1, P2]]),
                        _ap(cube, 0, [[T, P2], [tl, th], [1, tl]]),
                        mybir.AxisListType.XY,
                        AO.max,
                    )
                elif RED_XY and len(RED_SPLIT) == 3:
                    ta, tb, tc = RED_SPLIT
                    red = nc.vector.tensor_reduce(
                        _ap(img, (k % BUF) * P2, [[1, P2]]),
                        _ap(cube, 0, [[SIZE * T, SIZE], [T, SIZE], [tb * tc, ta], [tc, tb], [1, tc]]),
                        mybir.AxisListType.XYZ,
                        AO.max,
                    )
                elif RED_XY:
                    th, tl = RED_SPLIT
                    red = nc.vector.tensor_reduce(
                        _ap(img, (k % BUF) * P2, [[1, P2]]),
                        _ap(cube, 0, [[SIZE * T, SIZE], [T, SIZE], [tl, th], [1, tl]]),
                        mybir.AxisListType.XY,
                        AO.max,
                    )
                else:
                    red = nc.vector.tensor_reduce(
                        _ap(img, (k % BUF) * P2, [[1, P2]]),
                        _ap(cube, 0, [[SIZE * T, SIZE], [T, SIZE], [1, T]]),
                        mybir.AxisListType.X,
                        AO.max,
                    )
                if k > BUF - 1:
                    # WAR (BUF-rep slack): out-dma(k-BUF) must have read
                    # imgo[k%BUF]; red -> vrd -> Exp(k) orders it
                    red._wait_ge(dso, (k - BUF + 1) * 16)
                red.then_inc(vrd, 1)

        @block.scalar
        def _(scalar):
            for k in range(rep):
                o = (k % BUF) * P2
                if DUMMY_ACT:
                    # pad the ACT stream so the next Exp's vrd wait is already
                    # satisfied when reached (blocked waits stall globally)
                    nc.scalar.activation(
                        _ap(imgo, o, [[1, P2]]),
                        _ap(imgo, o, [[1, P2]]),
                        AF.Square,
                    )
                nc.scalar.activation(
                    _ap(imgo, o, [[1, P2]]),
                    _ap(img, o, [[1, P2]]),
                    AF.Exp,
                )._wait_ge(vrd, k + 1).then_inc(aex, 1)

    return nc


def make_in_maps(x: np.ndarray) -> list:
    """Shard x (1024, 64, 3) -> per-core host-prepped maps.

    Per core [128, 220] fp32: sqrt(g)*y[t] | sqrt(g)*x[t] | ln(I[t]) |
    sqrt(g)*grid, t innermost.
    """
    x = np.asarray(x, dtype=np.float32)
    maps = []
    sg = np.float32(np.sqrt(G))
    with np.errstate(divide="ignore"):
        lnI = np.log(x[:, :, 2]).astype(np.float32)  # (B, T); -inf ok
    for c in range(NCORES):
        sl = slice(c * BC, (c + 1) * BC)
        xc = np.empty((BC, XCOLS), np.float32)
        xc[:, 0:T] = sg * x[sl, :, 1]  # sqrt(g)*y
        xc[:, T : 2 * T] = sg * x[sl, :, 0]  # sqrt(g)*x
        xc[:, 2 * T : 3 * T] = lnI[sl]
        xc[:, 3 * T : 3 * T + SIZE] = sg * _GRID[None, :]
        xc[:, 3 * T + SIZE] = np.float32(np.e)
        maps.append({"xin": np.ascontiguousarray(xc)})
    return maps


def kernel(x: np.ndarray) -> np.ndarray:
    """Full inputs in, full output out: (1024, 64, 3) f32 -> (1024, 28, 28, 1) f32."""
    x = np.asarray(x, dtype=np.float32)
    assert x.shape == (B, T, 3), x.shape
    nc = build(rep=1)
    res = run_bass_kernel_spmd(nc, make_in_maps(x), list(range(NCORES)))
    outs = [res.results[c]["out"].reshape(BC, SIZE, SIZE, 1) for c in range(NCORES)]
    return np.concatenate(outs, axis=0)

